# revision 12
# baseline (speedup 1.0000x reference)
"""Trainium2 Bass kernel for nn_ClusterModel (MoE routing + segment pooling).

Model (see docstring math):
  xg = x[group_indices]                         # [4, N/4, 128] per-group gather
  h  = relu(xg @ W1[g] + b1[g])                 # [4, N/4, 1024]
  og = h @ W2[g] + b2[g]                        # [4, N/4, 512]
  new_feat = scatter(og) back to node order     # [N, 512]
  emb = segment_max(new_feat, fine clusters)    # [8192, 512]  (16 nodes/cluster)
  normed = InstanceNorm per coarse graph        # [8192, 512]  (256 clusters/graph)
  logits = normed @ w_out + b_out               # [8192, 16]

Sharding: 8 cores, each takes N/8 = 16384 consecutive nodes = 1024 fine
clusters = 4 coarse graphs.  All segment reductions are core-local (cores
split exactly at coarse-graph boundaries) -> zero collectives.

Per core the host gathers that core's x rows in (group, node)-sorted order
(transposed, zero-padded per group to a uniform capacity) so the device
GEMMs run over group-contiguous row blocks with replicated weights.  The
scatter-back + segment-max is done on device with dma_gather: og rows are
written to a DRAM scratch in sorted order and gathered back in
(cluster, member) order, then a pairwise max tree reduces the 16 members.
InstanceNorm stats + classifier run on-device in feature-major layout
(after a PE transpose of the pooled embeddings).

Matmuls run in float32r (rearranged fp32, full PE rate at N>=256).
"""

import numpy as np
from contextlib import ExitStack

import jax
import concourse.bass as bass
import concourse.tile as tile
from concourse import bacc, mybir
from concourse import bass2jax

F32 = mybir.dt.float32
F32R = mybir.dt.float32r
I16 = mybir.dt.int16
AF = mybir.ActivationFunctionType
ALU = mybir.AluOpType

# Problem constants (hardcoded per contest contract)
N = 131072
D = 128
KEXP = 1024
H = 512
NG = 4
F_SEG = 8192
G_SEG = 32
C_CLS = 16
EPS = 1e-5
NCORES = 8
P = 128
CHUNK = 512          # GEMM row-chunk
NEG = -3.0e38

_PROGRAM_CACHE: dict = {}


# ----------------------------------------------------------------------------
# Device program
# ----------------------------------------------------------------------------

def _build_program(GCAP: int, CCAP: int, MCAP: int, phases: int = 5):
    """Build the SPMD Bass program for given capacities.

    GCAP: padded rows per (core, group), multiple of CHUNK
    CCAP: padded clusters per (core, graph), multiple of 128
    MCAP: padded members per cluster, power of two
    phases: build only the first `phases` pipeline phases (debug bisection)
    """
    RTOT = NG * GCAP               # GEMM rows per core (padded)
    NROWS = 2 + RTOT               # og scratch rows (row0=zeros, row1=-inf)
    GPC = G_SEG // NCORES          # graphs per core = 4
    SLOTS = GPC * CCAP             # cluster slots per core
    NBLK = SLOTS // P              # gather blocks (128 clusters each)
    BPG = CCAP // P                # blocks per graph
    NCHUNK = GCAP // CHUNK
    KT = KEXP // P                 # 8 k-tiles in layer 2
    FT = H // P                    # 4 feature tiles of the 512-dim output

    nc = bacc.Bacc("TRN2", target_bir_lowering=False, debug=False,
                   num_devices=NCORES)

    xt_ap = nc.dram_tensor("xt", [P, RTOT], F32, kind="ExternalInput").ap()
    w1_ap = nc.dram_tensor("w1", [P, NG, KEXP], F32, kind="ExternalInput").ap()
    w2_ap = nc.dram_tensor("w2", [P, NG, KT, H], F32, kind="ExternalInput").ap()
    b1_ap = nc.dram_tensor("b1s", [P, NG * KT], F32, kind="ExternalInput").ap()
    b2_ap = nc.dram_tensor("b2r", [P, NG, H], F32, kind="ExternalInput").ap()
    wo_ap = nc.dram_tensor("wout", [P, FT, C_CLS], F32, kind="ExternalInput").ap()
    bo_ap = nc.dram_tensor("bout", [C_CLS, 1], F32, kind="ExternalInput").ap()
    ic_ap = nc.dram_tensor("invc", [P, GPC], F32, kind="ExternalInput").ap()
    gi_ap = nc.dram_tensor("gidx", [P, NBLK * MCAP * P // 16], I16,
                           kind="ExternalInput").ap()
    id_ap = nc.dram_tensor("ident", [P, P], F32, kind="ExternalInput").ap()
    og_ap = nc.dram_tensor("ogs", [NROWS, H], F32).ap()   # internal scratch
    lo_ap = nc.dram_tensor("logt", [C_CLS, SLOTS], F32, kind="ExternalOutput").ap()
    dbg_og_ap = dbg_emb_ap = None
    if phases <= 1:
        dbg_og_ap = nc.dram_tensor("dbg_og", [NROWS, H], F32,
                                   kind="ExternalOutput").ap()
    elif phases <= 3:
        dbg_emb_ap = nc.dram_tensor("dbg_emb", [P, NBLK, H], F32,
                                    kind="ExternalOutput").ap()

    with tile.TileContext(nc) as tc, ExitStack() as ctx:
        cst = ctx.enter_context(tc.tile_pool(name="cst", bufs=1))

        # --- resident constants -------------------------------------------
        w1_sb = cst.tile([P, NG, KEXP], F32R)
        w1_raw = cst.tile([P, NG, KEXP], F32)
        nc.sync.dma_start(out=w1_raw[:], in_=w1_ap[:])
        nc.vector.tensor_copy(w1_sb[:], w1_raw[:])        # round to f32r
        b1_sb = cst.tile([P, NG * KT], F32)
        nc.sync.dma_start(out=b1_sb[:], in_=b1_ap[:])
        b2_sb = cst.tile([P, NG, H], F32)
        nc.sync.dma_start(out=b2_sb[:], in_=b2_ap[:])
        wo_sb = cst.tile([P, FT, C_CLS], F32R)
        wo_raw = cst.tile([P, FT, C_CLS], F32)
        nc.sync.dma_start(out=wo_raw[:], in_=wo_ap[:])
        nc.vector.tensor_copy(wo_sb[:], wo_raw[:])
        bo_sb = cst.tile([C_CLS, 1], F32)
        nc.sync.dma_start(out=bo_sb[:], in_=bo_ap[:])
        ic_sb = cst.tile([P, GPC], F32)
        nc.sync.dma_start(out=ic_sb[:], in_=ic_ap[:])
        gi_sb = cst.tile([P, NBLK * MCAP * P // 16], I16)
        nc.sync.dma_start(out=gi_sb[:], in_=gi_ap[:])
        ident = cst.tile([P, P], F32)
        nc.sync.dma_start(out=ident[:], in_=id_ap[:])

        # og rows 0/1: zeros and -inf sentinels
        sent0 = cst.tile([1, H], F32)
        nc.vector.memset(sent0[:], 0.0)
        nc.sync.dma_start(out=og_ap[0:1, :], in_=sent0[:])
        sent1 = cst.tile([1, H], F32)
        nc.vector.memset(sent1[:], NEG)
        nc.sync.dma_start(out=og_ap[1:2, :], in_=sent1[:])

        emb_sb = cst.tile([P, NBLK, H], F32)    # pooled embeddings, cluster-major

        # --- phase 1: per-group MLP over row chunks ------------------------
        with tc.tile_pool(name="g_sb", bufs=2) as gsb, \
             tc.tile_pool(name="g_w2", bufs=2) as gw2, \
             tc.tile_pool(name="g_ht", bufs=3) as ght, \
             tc.tile_pool(name="g_og", bufs=6) as gog, \
             tc.tile_pool(name="g_ph", bufs=2, space="PSUM") as gph, \
             tc.tile_pool(name="g_po", bufs=4, space="PSUM") as gpo:
            for g in range(NG):
                w2_sb = gw2.tile([P, KT, H], F32R, tag="w2")
                w2_raw = gw2.tile([P, KT, H], F32, tag="w2raw")
                nc.sync.dma_start(out=w2_raw[:], in_=w2_ap[:, g, :, :])
                nc.vector.tensor_copy(w2_sb[:], w2_raw[:])
                for j in range(NCHUNK):
                    col0 = g * GCAP + j * CHUNK
                    xt_sb = gsb.tile([P, CHUNK], F32, tag="xt")
                    nc.sync.dma_start(out=xt_sb[:],
                                      in_=xt_ap[:, col0:col0 + CHUNK])
                    xt_r = gsb.tile([P, CHUNK], F32R, tag="xtr")
                    nc.scalar.activation(xt_r[:], xt_sb[:], AF.Copy)

                    og_ps = [gpo.tile([P, H], F32, tag="og", name=f"og_ps{s}")
                             for s in range(4)]
                    for kt in range(KT):
                        h_ps = gph.tile([P, CHUNK], F32, tag="h")
                        nc.tensor.matmul(
                            h_ps[:], w1_sb[:, g, kt * P:(kt + 1) * P],
                            xt_r[:], start=True, stop=True)
                        ht_sb = ght.tile([P, CHUNK], F32R, tag="ht")
                        # relu(h + b1), rounded to f32r
                        if kt % 2 == 0:
                            nc.scalar.activation(
                                ht_sb[:], h_ps[:], AF.Relu,
                                bias=b1_sb[:, g * KT + kt:g * KT + kt + 1])
                        else:
                            nc.vector.tensor_scalar(
                                ht_sb[:], h_ps[:],
                                b1_sb[:, g * KT + kt:g * KT + kt + 1], 0.0,
                                op0=ALU.add, op1=ALU.max)
                        for s in range(4):
                            nc.tensor.matmul(
                                og_ps[s][:],
                                ht_sb[:, s * P:(s + 1) * P],
                                w2_sb[:, kt, :],
                                start=(kt == 0), stop=(kt == KT - 1))
                    for s in range(4):
                        og_sb = gog.tile([P, H], F32, tag="og")
                        nc.vector.tensor_tensor(
                            out=og_sb[:], in0=og_ps[s][:], in1=b2_sb[:, g, :],
                            op=ALU.add)
                        r0 = 2 + col0 + s * P
                        nc.sync.dma_start(out=og_ap[r0:r0 + P, :], in_=og_sb[:])

        if dbg_og_ap is not None:
            nc.sync.dma_start(out=dbg_og_ap[:], in_=og_ap[:])

        # --- phase 2: gather to (cluster, member) order + max tree ---------
        PH = H // 2  # half-row gather (SBUF economy)
        with tc.tile_pool(name="p2", bufs=2) as p2, \
             tc.tile_pool(name="p2t", bufs=2) as p2t:
            for t in range(NBLK if phases >= 2 else 0):
                idx_sl = gi_sb[:, t * (MCAP * P // 16):(t + 1) * (MCAP * P // 16)]
                for hh in range(2):
                    gat = p2.tile([P, MCAP, PH], F32, tag="gat")
                    nc.gpsimd.dma_gather(
                        gat[:], og_ap[:, hh * PH:(hh + 1) * PH], idx_sl,
                        MCAP * P, MCAP * P, PH, elem_step=H,
                        single_packet=False)
                    # pairwise max tree over members
                    cur = gat
                    m = MCAP
                    while m > 1:
                        m //= 2
                        if m == 1:
                            nxt_ap = emb_sb[:, t, hh * PH:(hh + 1) * PH]
                            nc.vector.tensor_tensor(
                                out=nxt_ap, in0=cur[:, 0:1, :].opt({0}),
                                in1=cur[:, 1:2, :].opt({0}), op=ALU.max)
                        else:
                            nxt = p2t.tile([P, m, PH], F32, tag=f"tm{m}")
                            nc.vector.tensor_tensor(
                                out=nxt[:], in0=cur[:, 0:m, :],
                                in1=cur[:, m:2 * m, :], op=ALU.max)
                            cur = nxt

        if dbg_emb_ap is not None:
            dbg_emb_sb_done = cst.tile([1, 1], F32)  # noqa: F841 unused marker
            nc.sync.dma_start(out=dbg_emb_ap[:], in_=emb_sb[:])

        # --- phase 3: transpose emb -> feature-major -----------------------
        embt = [cst.tile([P, GPC, CCAP], F32, tag=f"embt{f}", name=f"embt{f}") for f in range(FT)]
        with tc.tile_pool(name="p3", bufs=4, space="PSUM") as p3ps:
            for t in range(NBLK if phases >= 3 else 0):
                gi_, bi_ = t // BPG, t % BPG
                for f in range(FT):
                    tp = p3ps.tile([P, P], F32, tag="tp")
                    nc.tensor.transpose(
                        tp[:], emb_sb[:, t, f * P:(f + 1) * P], ident[:])
                    nc.scalar.activation(
                        embt[f][:, gi_, bi_ * P:(bi_ + 1) * P], tp[:], AF.Copy)

        # --- phase 4: instance norm (per graph, per channel) ---------------
        embn = [cst.tile([P, GPC, CCAP], F32R, tag=f"embn{f}", name=f"embn{f}") for f in range(FT)]
        with tc.tile_pool(name="p4", bufs=8) as p4:
            for f in range(FT if phases >= 4 else 0):
                sm = p4.tile([P, GPC], F32, tag="sm")
                nc.vector.tensor_reduce(sm[:], embt[f][:], mybir.AxisListType.X,
                                        ALU.add)
                sq = p4.tile([P, GPC, CCAP], F32, tag="sq")
                nc.scalar.activation(sq[:], embt[f][:], AF.Square)
                s2 = p4.tile([P, GPC], F32, tag="s2")
                nc.vector.tensor_reduce(s2[:], sq[:], mybir.AxisListType.X,
                                        ALU.add)
                mean = p4.tile([P, GPC], F32, tag="mean")
                nc.vector.tensor_tensor(out=mean[:], in0=sm[:], in1=ic_sb[:],
                                        op=ALU.mult)
                ex2 = p4.tile([P, GPC], F32, tag="ex2")
                nc.vector.tensor_tensor(out=ex2[:], in0=s2[:], in1=ic_sb[:],
                                        op=ALU.mult)
                m2 = p4.tile([P, GPC], F32, tag="m2")
                nc.vector.tensor_tensor(out=m2[:], in0=mean[:], in1=mean[:],
                                        op=ALU.mult)
                var = p4.tile([P, GPC], F32, tag="var")
                nc.vector.tensor_tensor(out=var[:], in0=ex2[:], in1=m2[:],
                                        op=ALU.subtract)
                ve = p4.tile([P, GPC], F32, tag="ve")
                nc.vector.tensor_scalar_add(ve[:], var[:], EPS)
                sd = p4.tile([P, GPC], F32, tag="sd")
                nc.scalar.activation(sd[:], ve[:], AF.Sqrt)
                rstd = p4.tile([P, GPC], F32, tag="rstd")
                nc.vector.reciprocal(rstd[:], sd[:])
                for gi_ in range(GPC):
                    nc.vector.tensor_scalar(
                        embn[f][:, gi_, :], embt[f][:, gi_, :],
                        mean[:, gi_:gi_ + 1], rstd[:, gi_:gi_ + 1],
                        op0=ALU.subtract, op1=ALU.mult)

        # --- phase 5: classifier ------------------------------------------
        NSL = 512
        with tc.tile_pool(name="p5", bufs=2) as p5, \
             tc.tile_pool(name="p5ps", bufs=2, space="PSUM") as p5ps:
            for n0 in (range(0, SLOTS, NSL) if phases >= 5 else []):
                nw = min(NSL, SLOTS - n0)
                lg_ps = p5ps.tile([C_CLS, NSL], F32, tag="lg")
                for f in range(FT):
                    rhs = embn[f].rearrange("p g c -> p (g c)")[:, n0:n0 + nw]
                    nc.tensor.matmul(lg_ps[:, :nw], wo_sb[:, f, :], rhs,
                                     start=(f == 0), stop=(f == FT - 1))
                lg_sb = p5.tile([C_CLS, NSL], F32, tag="lgs")
                nc.vector.tensor_scalar(lg_sb[:, :nw], lg_ps[:, :nw],
                                        bo_sb[:], None, op0=ALU.add)
                nc.sync.dma_start(out=lo_ap[:, n0:n0 + nw], in_=lg_sb[:, :nw])

    nc.compile()
    return nc


# ----------------------------------------------------------------------------
# PJRT runner (mirrors bass2jax.run_bass_via_pjrt, but reusable for timing)
# ----------------------------------------------------------------------------

class _Runner:
    def __init__(self, nc):
        from jax.sharding import Mesh, PartitionSpec
        from jax.experimental.shard_map import shard_map

        bass2jax.install_neuronx_cc_hook()
        self.nc = nc
        part_name = (nc.partition_id_tensor.name
                     if nc.partition_id_tensor else None)
        in_names, out_names, out_avals, zero_outs = [], [], [], []
        for alloc in nc.m.functions[0].allocations:
            if not isinstance(alloc, mybir.MemoryLocationSet):
                continue
            name = alloc.memorylocations[0].name
            if alloc.kind == "ExternalInput":
                if name != part_name:
                    in_names.append(name)
            elif alloc.kind == "ExternalOutput":
                out_names.append(name)
                shape = tuple(alloc.tensor_shape)
                dtype = mybir.dt.np(alloc.dtype)
                out_avals.append(jax.core.ShapedArray(shape, dtype))
                zero_outs.append(np.zeros(shape, dtype))
        self.n_params = len(in_names)
        self.in_names = in_names + out_names
        if part_name is not None:
            self.in_names = self.in_names + [part_name]
        self.out_names = out_names
        self.out_avals = out_avals
        self.zero_outs = zero_outs

        def _body(*args):
            operands = list(args)
            if part_name is not None:
                operands.append(bass2jax.partition_id_tensor())
            outs = bass2jax._bass_exec_p.bind(
                *operands,
                out_avals=tuple(out_avals),
                in_names=tuple(self.in_names),
                out_names=tuple(out_names),
                lowering_input_output_aliases=(),
                sim_require_finite=True,
                sim_require_nnan=True,
                nc=nc,
            )
            return tuple(outs)

        devices = jax.devices()[:NCORES]
        self.mesh = Mesh(np.asarray(devices), ("core",))
        n_all = self.n_params + len(out_names)
        self.fn = jax.jit(
            shard_map(_body, mesh=self.mesh,
                      in_specs=(PartitionSpec("core"),) * n_all,
                      out_specs=(PartitionSpec("core"),) * len(out_names),
                      check_rep=False),
            keep_unused=True,
        )

    def prepare(self, in_maps):
        concat = [
            np.concatenate([np.asarray(m[nm]) for m in in_maps], axis=0)
            for nm in self.in_names[:self.n_params]
        ]
        concat += [
            np.zeros((NCORES * z.shape[0], *z.shape[1:]), z.dtype)
            for z in self.zero_outs
        ]
        return concat

    def run(self, args):
        outs = self.fn(*args)
        return [
            {nm: np.asarray(outs[i]).reshape(NCORES, *self.out_avals[i].shape)[c]
             for i, nm in enumerate(self.out_names)}
            for c in range(NCORES)
        ]


# ----------------------------------------------------------------------------
# Host-side sharding / index plumbing
# ----------------------------------------------------------------------------

def _round_up(v, m):
    return (v + m - 1) // m * m


def _pow2_round(v):
    p = 1
    while p < v:
        p *= 2
    return p


def prepare(x, group_indices, pool_cluster_fine, batch_cluster_coarse,
            W1, b1, W2, b2, w_out, b_out):
    """Compute capacities + per-core input maps. Returns (key, in_maps, meta)."""
    x = np.asarray(x)
    gidx = np.asarray(group_indices)
    pcf = np.asarray(pool_cluster_fine).astype(np.int64)
    bcc = np.asarray(batch_cluster_coarse).astype(np.int64)
    W1 = np.asarray(W1, dtype=np.float32)
    b1 = np.asarray(b1, dtype=np.float32)
    W2 = np.asarray(W2, dtype=np.float32)
    b2 = np.asarray(b2, dtype=np.float32)
    w_out = np.asarray(w_out, dtype=np.float32)
    b_out = np.asarray(b_out, dtype=np.float32)

    GPC = G_SEG // NCORES

    # node -> group (later groups win on duplicates, matching scatter order)
    gid = np.full(N, -1, np.int32)
    for g in range(NG):
        gid[gidx[g]] = g

    # graph/cluster/node boundaries (general sorted-segment support)
    fine_lo = np.searchsorted(bcc, np.arange(0, G_SEG, GPC))          # per core
    fine_hi = np.searchsorted(bcc, np.arange(GPC - 1, G_SEG, GPC), "right")
    node_lo = np.searchsorted(pcf, fine_lo)
    node_hi = np.searchsorted(pcf, fine_hi)

    # cluster boundaries for every fine cluster
    cl_lo = np.searchsorted(pcf, np.arange(F_SEG))
    cl_hi = np.searchsorted(pcf, np.arange(F_SEG), "right")
    cl_sz = cl_hi - cl_lo
    MCAP = _pow2_round(max(1, int(cl_sz.max())))

    # graph boundaries in cluster space, per core
    g_lo = np.searchsorted(bcc, np.arange(G_SEG))
    g_hi = np.searchsorted(bcc, np.arange(G_SEG), "right")
    g_sz = g_hi - g_lo
    CCAP = _round_up(max(1, int(g_sz.max())), P)

    # rows per (core, group)
    counts = np.zeros((NCORES, NG), np.int64)
    core_nodes = []
    for c in range(NCORES):
        nd = np.arange(node_lo[c], node_hi[c])
        core_nodes.append(nd)
        gs = gid[nd]
        for g in range(NG):
            counts[c, g] = int((gs == g).sum())
    GCAP = _round_up(max(1, int(counts.max())), CHUNK)
    RTOT = NG * GCAP
    assert 2 + RTOT < 32768, f"GCAP={GCAP} too large for int16 gather indices"
    SLOTS = GPC * CCAP
    NBLK = SLOTS // P

    # replicated weight prep (shared across cores)
    w1_h = np.ascontiguousarray(W1.transpose(1, 0, 2))                 # [128,4,1024]
    w2_h = np.ascontiguousarray(
        W2.reshape(NG, KEXP // P, P, H).transpose(2, 0, 1, 3))         # [128,4,8,512]
    b1_h = np.ascontiguousarray(
        b1.reshape(NG, KEXP // P, P).transpose(2, 0, 1).reshape(P, -1))  # [128,32]
    b2_h = np.ascontiguousarray(
        np.broadcast_to(b2[None, :, :], (P, NG, H)))                   # [128,4,512]
    wo_h = np.ascontiguousarray(
        w_out.reshape(H // P, P, C_CLS).transpose(1, 0, 2))            # [128,4,16]
    bo_h = np.ascontiguousarray(b_out.reshape(C_CLS, 1))               # [16,1]

    in_maps = []
    meta = []
    for c in range(NCORES):
        nd = core_nodes[c]
        gs = gid[nd]
        xt = np.zeros((P, RTOT), np.float32)
        rows = np.zeros(N, np.int32)     # node -> og row (0 = zero row)
        for g in range(NG):
            sel = nd[gs == g]
            cnt = len(sel)
            xt[:, g * GCAP:g * GCAP + cnt] = x[sel].T
            rows[sel] = 2 + g * GCAP + np.arange(cnt, dtype=np.int32)

        # member table: [SLOTS, MCAP] og-row indices
        member = np.ones((SLOTS, MCAP), np.int32)      # 1 = -inf row (pad member)
        clusters_c = np.arange(fine_lo[c], fine_hi[c])
        inv_cnt = np.zeros(GPC, np.float32)
        for gi in range(GPC):
            gg = c * GPC + gi
            n_cl = int(g_sz[gg])
            inv_cnt[gi] = 1.0 / max(n_cl, 1)
        for s, f in enumerate(clusters_c):
            gi = int(bcc[f]) - c * GPC
            j = f - g_lo[int(bcc[f])]
            slot = gi * CCAP + j
            sz = int(cl_sz[f])
            member[slot, :sz] = rows[pcf_nodes_start(cl_lo, f) +
                                     np.arange(sz)]
        # pad clusters: all-zero row (emb = 0 so norm sums are unaffected)
        used = np.zeros(SLOTS, bool)
        for f in clusters_c:
            gi = int(bcc[f]) - c * GPC
            slot = gi * CCAP + (f - g_lo[int(bcc[f])])
            used[slot] = True
        member[~used, :] = 0

        # wrap indices for dma_gather: per block t, seq i = m*128 + a
        gidx_w = np.zeros((P, NBLK * MCAP * P // 16), np.int16)
        for t in range(NBLK):
            mt = member[t * P:(t + 1) * P]               # [128, MCAP]
            seq = mt.T.reshape(-1)                        # i = m*128 + a
            w = seq.reshape(-1, 16).T.astype(np.int16)    # [16, MCAP*128/16]
            gidx_w[:, t * (MCAP * P // 16):(t + 1) * (MCAP * P // 16)] = \
                np.tile(w, (8, 1))

        in_maps.append({
            "xt": xt,
            "w1": w1_h, "w2": w2_h, "b1s": b1_h, "b2r": b2_h,
            "wout": wo_h, "bout": bo_h,
            "invc": np.broadcast_to(inv_cnt[None, :], (P, GPC)).copy(),
            "ident": np.eye(P, dtype=np.float32),
            "gidx": gidx_w,
        })
        meta.append({"clusters": clusters_c, "fine_lo": int(fine_lo[c]),
                     "g_lo": g_lo, "c": c})

    key = (GCAP, CCAP, MCAP)
    return key, in_maps, meta, (CCAP,)


def pcf_nodes_start(cl_lo, f):
    return int(cl_lo[f])


def get_runner(key):
    if key not in _PROGRAM_CACHE:
        nc = _build_program(*key)
        _PROGRAM_CACHE[key] = _Runner(nc)
    return _PROGRAM_CACHE[key]


def kernel(**inputs) -> np.ndarray:
    key, in_maps, meta, (CCAP,) = prepare(**inputs)
    runner = get_runner(key)
    args = runner.prepare(in_maps)
    results = runner.run(args)

    bcc = np.asarray(inputs["batch_cluster_coarse"]).astype(np.int64)
    GPC = G_SEG // NCORES
    g_lo = np.searchsorted(bcc, np.arange(G_SEG))
    out = np.zeros((F_SEG, C_CLS), np.float32)
    for c in range(NCORES):
        lo = results[c]["logt"]              # [16, SLOTS]
        for f in meta[c]["clusters"]:
            gi = int(bcc[f]) - c * GPC
            slot = gi * CCAP + (int(f) - int(g_lo[int(bcc[f])]))
            out[f] = lo[:, slot]
    return out


# revision 13
# speedup vs baseline: 88.4209x; 88.4209x over previous
"""Trainium2 Bass kernel for nn_ClusterModel (MoE routing + segment pooling).

Model (see docstring math):
  xg = x[group_indices]                         # [4, N/4, 128] per-group gather
  h  = relu(xg @ W1[g] + b1[g])                 # [4, N/4, 1024]
  og = h @ W2[g] + b2[g]                        # [4, N/4, 512]
  new_feat = scatter(og) back to node order     # [N, 512]
  emb = segment_max(new_feat, fine clusters)    # [8192, 512]  (16 nodes/cluster)
  normed = InstanceNorm per coarse graph        # [8192, 512]  (256 clusters/graph)
  logits = normed @ w_out + b_out               # [8192, 16]

Sharding: 8 cores, each takes N/8 = 16384 consecutive nodes = 1024 fine
clusters = 4 coarse graphs.  All segment reductions are core-local (cores
split exactly at coarse-graph boundaries) -> zero collectives.

Per core the host gathers that core's x rows in (group, node)-sorted order
(transposed, zero-padded per group to a uniform capacity) so the device
GEMMs run over group-contiguous row blocks with replicated weights.  The
scatter-back + segment-max is done on device with dma_gather: og rows are
written to a DRAM scratch in sorted order and gathered back in
(cluster, member) order, then a pairwise max tree reduces the 16 members.
InstanceNorm stats + classifier run on-device in feature-major layout
(after a PE transpose of the pooled embeddings).

Matmuls run in float32r (rearranged fp32, full PE rate at N>=256).
"""

import numpy as np
from contextlib import ExitStack

import jax
import concourse.bass as bass
import concourse.tile as tile
from concourse import bacc, mybir
from concourse import bass2jax

F32 = mybir.dt.float32
F32R = mybir.dt.float32r
I16 = mybir.dt.int16
AF = mybir.ActivationFunctionType
ALU = mybir.AluOpType

# Problem constants (hardcoded per contest contract)
N = 131072
D = 128
KEXP = 1024
H = 512
NG = 4
F_SEG = 8192
G_SEG = 32
C_CLS = 16
EPS = 1e-5
NCORES = 8
P = 128
CHUNK = 512          # GEMM row-chunk
NEG = -3.0e38

_PROGRAM_CACHE: dict = {}


# ----------------------------------------------------------------------------
# Device program
# ----------------------------------------------------------------------------

def _build_program(GCAP: int, CCAP: int, MCAP: int, phases: int = 5,
                   repeat: int = 1):
    """Build the SPMD Bass program for given capacities.

    GCAP: padded rows per (core, group), multiple of CHUNK
    CCAP: padded clusters per (core, graph), multiple of 128
    MCAP: padded members per cluster, power of two
    phases: build only the first `phases` pipeline phases (debug bisection)
    """
    RTOT = NG * GCAP               # GEMM rows per core (padded)
    NROWS = 2 + RTOT               # og scratch rows (row0=zeros, row1=-inf)
    GPC = G_SEG // NCORES          # graphs per core = 4
    SLOTS = GPC * CCAP             # cluster slots per core
    NBLK = SLOTS // P              # gather blocks (128 clusters each)
    BPG = CCAP // P                # blocks per graph
    NCHUNK = GCAP // CHUNK
    KT = KEXP // P                 # 8 k-tiles in layer 2
    FT = H // P                    # 4 feature tiles of the 512-dim output

    nc = bacc.Bacc("TRN2", target_bir_lowering=False, debug=False,
                   num_devices=NCORES)

    xt_ap = nc.dram_tensor("xt", [P, RTOT], F32, kind="ExternalInput").ap()
    w1_ap = nc.dram_tensor("w1", [P, NG, KEXP], F32, kind="ExternalInput").ap()
    w2_ap = nc.dram_tensor("w2", [P, NG, KT, H], F32, kind="ExternalInput").ap()
    b1_ap = nc.dram_tensor("b1s", [P, NG * KT], F32, kind="ExternalInput").ap()
    b2_ap = nc.dram_tensor("b2r", [P, NG, H], F32, kind="ExternalInput").ap()
    wo_ap = nc.dram_tensor("wout", [P, FT, C_CLS], F32, kind="ExternalInput").ap()
    bo_ap = nc.dram_tensor("bout", [C_CLS, 1], F32, kind="ExternalInput").ap()
    ic_ap = nc.dram_tensor("invc", [P, GPC], F32, kind="ExternalInput").ap()
    gi_ap = nc.dram_tensor("gidx", [P, NBLK * MCAP * P // 16], I16,
                           kind="ExternalInput").ap()
    id_ap = nc.dram_tensor("ident", [P, P], F32, kind="ExternalInput").ap()
    og_ap = nc.dram_tensor("ogs", [NROWS, H], F32).ap()   # internal scratch
    lo_ap = nc.dram_tensor("logt", [C_CLS, SLOTS], F32, kind="ExternalOutput").ap()
    dbg_og_ap = dbg_emb_ap = None
    if phases <= 1:
        dbg_og_ap = nc.dram_tensor("dbg_og", [NROWS, H], F32,
                                   kind="ExternalOutput").ap()
    elif phases <= 3:
        dbg_emb_ap = nc.dram_tensor("dbg_emb", [P, NBLK, H], F32,
                                    kind="ExternalOutput").ap()

    with tile.TileContext(nc) as tc, ExitStack() as ctx:
        cst = ctx.enter_context(tc.tile_pool(name="cst", bufs=1))

        # --- resident constants -------------------------------------------
        w1_sb = cst.tile([P, NG, KEXP], F32R)
        w1_raw = cst.tile([P, NG, KEXP], F32)
        nc.sync.dma_start(out=w1_raw[:], in_=w1_ap[:])
        nc.vector.tensor_copy(w1_sb[:], w1_raw[:])        # round to f32r
        b1_sb = cst.tile([P, NG * KT], F32)
        nc.sync.dma_start(out=b1_sb[:], in_=b1_ap[:])
        b2_sb = cst.tile([P, NG, H], F32)
        nc.sync.dma_start(out=b2_sb[:], in_=b2_ap[:])
        wo_sb = cst.tile([P, FT, C_CLS], F32R)
        wo_raw = cst.tile([P, FT, C_CLS], F32)
        nc.sync.dma_start(out=wo_raw[:], in_=wo_ap[:])
        nc.vector.tensor_copy(wo_sb[:], wo_raw[:])
        bo_sb = cst.tile([C_CLS, 1], F32)
        nc.sync.dma_start(out=bo_sb[:], in_=bo_ap[:])
        ic_sb = cst.tile([P, GPC], F32)
        nc.sync.dma_start(out=ic_sb[:], in_=ic_ap[:])
        gi_sb = cst.tile([P, NBLK * MCAP * P // 16], I16)
        nc.sync.dma_start(out=gi_sb[:], in_=gi_ap[:])
        ident = cst.tile([P, P], F32)
        nc.sync.dma_start(out=ident[:], in_=id_ap[:])

        # og rows 0/1: zeros and -inf sentinels
        sent0 = cst.tile([1, H], F32)
        nc.vector.memset(sent0[:], 0.0)
        nc.sync.dma_start(out=og_ap[0:1, :], in_=sent0[:])
        sent1 = cst.tile([1, H], F32)
        nc.vector.memset(sent1[:], NEG)
        nc.sync.dma_start(out=og_ap[1:2, :], in_=sent1[:])

        emb_sb = cst.tile([P, NBLK, H], F32)    # pooled embeddings, cluster-major

        rep_cm = tc.For_i(0, repeat, 1) if repeat > 1 else None
        if rep_cm is not None:
            ctx.enter_context(rep_cm)

        # --- phase 1: per-group MLP over row chunks ------------------------
        with tc.tile_pool(name="g_sb", bufs=2) as gsb, \
             tc.tile_pool(name="g_w2", bufs=2) as gw2, \
             tc.tile_pool(name="g_ht", bufs=3) as ght, \
             tc.tile_pool(name="g_og", bufs=6) as gog, \
             tc.tile_pool(name="g_ph", bufs=2, space="PSUM") as gph, \
             tc.tile_pool(name="g_po", bufs=4, space="PSUM") as gpo:
            for g in range(NG):
                w2_sb = gw2.tile([P, KT, H], F32R, tag="w2")
                w2_raw = gw2.tile([P, KT, H], F32, tag="w2raw")
                nc.sync.dma_start(out=w2_raw[:], in_=w2_ap[:, g, :, :])
                nc.vector.tensor_copy(w2_sb[:], w2_raw[:])
                for j in range(NCHUNK):
                    col0 = g * GCAP + j * CHUNK
                    xt_sb = gsb.tile([P, CHUNK], F32, tag="xt")
                    nc.sync.dma_start(out=xt_sb[:],
                                      in_=xt_ap[:, col0:col0 + CHUNK])
                    xt_r = gsb.tile([P, CHUNK], F32R, tag="xtr")
                    nc.scalar.activation(xt_r[:], xt_sb[:], AF.Copy)

                    og_ps = [gpo.tile([P, H], F32, tag="og", name=f"og_ps{s}")
                             for s in range(4)]
                    for kt in range(KT):
                        h_ps = gph.tile([P, CHUNK], F32, tag="h")
                        nc.tensor.matmul(
                            h_ps[:], w1_sb[:, g, kt * P:(kt + 1) * P],
                            xt_r[:], start=True, stop=True)
                        ht_sb = ght.tile([P, CHUNK], F32R, tag="ht")
                        # relu(h + b1), rounded to f32r
                        if kt % 2 == 0:
                            nc.scalar.activation(
                                ht_sb[:], h_ps[:], AF.Relu,
                                bias=b1_sb[:, g * KT + kt:g * KT + kt + 1])
                        else:
                            nc.vector.tensor_scalar(
                                ht_sb[:], h_ps[:],
                                b1_sb[:, g * KT + kt:g * KT + kt + 1], 0.0,
                                op0=ALU.add, op1=ALU.max)
                        for s in range(4):
                            nc.tensor.matmul(
                                og_ps[s][:],
                                ht_sb[:, s * P:(s + 1) * P],
                                w2_sb[:, kt, :],
                                start=(kt == 0), stop=(kt == KT - 1))
                    for s in range(4):
                        og_sb = gog.tile([P, H], F32, tag="og")
                        nc.vector.tensor_tensor(
                            out=og_sb[:], in0=og_ps[s][:], in1=b2_sb[:, g, :],
                            op=ALU.add)
                        r0 = 2 + col0 + s * P
                        nc.sync.dma_start(out=og_ap[r0:r0 + P, :], in_=og_sb[:])

        if dbg_og_ap is not None:
            nc.sync.dma_start(out=dbg_og_ap[:], in_=og_ap[:])

        # --- phase 2: gather to (cluster, member) order + max tree ---------
        PH = H // 2  # half-row gather (SBUF economy)
        with tc.tile_pool(name="p2", bufs=2) as p2, \
             tc.tile_pool(name="p2t", bufs=2) as p2t:
            for t in range(NBLK if phases >= 2 else 0):
                idx_sl = gi_sb[:, t * (MCAP * P // 16):(t + 1) * (MCAP * P // 16)]
                for hh in range(2):
                    gat = p2.tile([P, MCAP, PH], F32, tag="gat")
                    nc.gpsimd.dma_gather(
                        gat[:], og_ap[:, hh * PH:(hh + 1) * PH], idx_sl,
                        MCAP * P, MCAP * P, PH, elem_step=H,
                        single_packet=False)
                    # pairwise max tree over members
                    cur = gat
                    m = MCAP
                    while m > 1:
                        m //= 2
                        if m == 1:
                            nxt_ap = emb_sb[:, t, hh * PH:(hh + 1) * PH]
                            nc.vector.tensor_tensor(
                                out=nxt_ap, in0=cur[:, 0:1, :].opt({0}),
                                in1=cur[:, 1:2, :].opt({0}), op=ALU.max)
                        else:
                            nxt = p2t.tile([P, m, PH], F32, tag=f"tm{m}")
                            nc.vector.tensor_tensor(
                                out=nxt[:], in0=cur[:, 0:m, :],
                                in1=cur[:, m:2 * m, :], op=ALU.max)
                            cur = nxt

        if dbg_emb_ap is not None:
            dbg_emb_sb_done = cst.tile([1, 1], F32)  # noqa: F841 unused marker
            nc.sync.dma_start(out=dbg_emb_ap[:], in_=emb_sb[:])

        # --- phase 3: transpose emb -> feature-major -----------------------
        embt = [cst.tile([P, GPC, CCAP], F32, tag=f"embt{f}", name=f"embt{f}") for f in range(FT)]
        with tc.tile_pool(name="p3", bufs=4, space="PSUM") as p3ps:
            for t in range(NBLK if phases >= 3 else 0):
                gi_, bi_ = t // BPG, t % BPG
                for f in range(FT):
                    tp = p3ps.tile([P, P], F32, tag="tp")
                    nc.tensor.transpose(
                        tp[:], emb_sb[:, t, f * P:(f + 1) * P], ident[:])
                    nc.scalar.activation(
                        embt[f][:, gi_, bi_ * P:(bi_ + 1) * P], tp[:], AF.Copy)

        # --- phase 4: instance norm (per graph, per channel) ---------------
        embn = [cst.tile([P, GPC, CCAP], F32R, tag=f"embn{f}", name=f"embn{f}") for f in range(FT)]
        with tc.tile_pool(name="p4", bufs=8) as p4:
            for f in range(FT if phases >= 4 else 0):
                sm = p4.tile([P, GPC], F32, tag="sm")
                nc.vector.tensor_reduce(sm[:], embt[f][:], mybir.AxisListType.X,
                                        ALU.add)
                sq = p4.tile([P, GPC, CCAP], F32, tag="sq")
                nc.scalar.activation(sq[:], embt[f][:], AF.Square)
                s2 = p4.tile([P, GPC], F32, tag="s2")
                nc.vector.tensor_reduce(s2[:], sq[:], mybir.AxisListType.X,
                                        ALU.add)
                mean = p4.tile([P, GPC], F32, tag="mean")
                nc.vector.tensor_tensor(out=mean[:], in0=sm[:], in1=ic_sb[:],
                                        op=ALU.mult)
                ex2 = p4.tile([P, GPC], F32, tag="ex2")
                nc.vector.tensor_tensor(out=ex2[:], in0=s2[:], in1=ic_sb[:],
                                        op=ALU.mult)
                m2 = p4.tile([P, GPC], F32, tag="m2")
                nc.vector.tensor_tensor(out=m2[:], in0=mean[:], in1=mean[:],
                                        op=ALU.mult)
                var = p4.tile([P, GPC], F32, tag="var")
                nc.vector.tensor_tensor(out=var[:], in0=ex2[:], in1=m2[:],
                                        op=ALU.subtract)
                ve = p4.tile([P, GPC], F32, tag="ve")
                nc.vector.tensor_scalar_add(ve[:], var[:], EPS)
                sd = p4.tile([P, GPC], F32, tag="sd")
                nc.scalar.activation(sd[:], ve[:], AF.Sqrt)
                rstd = p4.tile([P, GPC], F32, tag="rstd")
                nc.vector.reciprocal(rstd[:], sd[:])
                for gi_ in range(GPC):
                    nc.vector.tensor_scalar(
                        embn[f][:, gi_, :], embt[f][:, gi_, :],
                        mean[:, gi_:gi_ + 1], rstd[:, gi_:gi_ + 1],
                        op0=ALU.subtract, op1=ALU.mult)

        # --- phase 5: classifier ------------------------------------------
        NSL = 512
        with tc.tile_pool(name="p5", bufs=2) as p5, \
             tc.tile_pool(name="p5ps", bufs=2, space="PSUM") as p5ps:
            for n0 in (range(0, SLOTS, NSL) if phases >= 5 else []):
                nw = min(NSL, SLOTS - n0)
                lg_ps = p5ps.tile([C_CLS, NSL], F32, tag="lg")
                for f in range(FT):
                    rhs = embn[f].rearrange("p g c -> p (g c)")[:, n0:n0 + nw]
                    nc.tensor.matmul(lg_ps[:, :nw], wo_sb[:, f, :], rhs,
                                     start=(f == 0), stop=(f == FT - 1))
                lg_sb = p5.tile([C_CLS, NSL], F32, tag="lgs")
                nc.vector.tensor_scalar(lg_sb[:, :nw], lg_ps[:, :nw],
                                        bo_sb[:], None, op0=ALU.add)
                nc.sync.dma_start(out=lo_ap[:, n0:n0 + nw], in_=lg_sb[:, :nw])

    nc.compile()
    return nc


# ----------------------------------------------------------------------------
# PJRT runner (mirrors bass2jax.run_bass_via_pjrt, but reusable for timing)
# ----------------------------------------------------------------------------

class _Runner:
    def __init__(self, nc):
        from jax.sharding import Mesh, PartitionSpec
        from jax.experimental.shard_map import shard_map

        bass2jax.install_neuronx_cc_hook()
        self.nc = nc
        part_name = (nc.partition_id_tensor.name
                     if nc.partition_id_tensor else None)
        in_names, out_names, out_avals, zero_outs = [], [], [], []
        for alloc in nc.m.functions[0].allocations:
            if not isinstance(alloc, mybir.MemoryLocationSet):
                continue
            name = alloc.memorylocations[0].name
            if alloc.kind == "ExternalInput":
                if name != part_name:
                    in_names.append(name)
            elif alloc.kind == "ExternalOutput":
                out_names.append(name)
                shape = tuple(alloc.tensor_shape)
                dtype = mybir.dt.np(alloc.dtype)
                out_avals.append(jax.core.ShapedArray(shape, dtype))
                zero_outs.append(np.zeros(shape, dtype))
        self.n_params = len(in_names)
        self.in_names = in_names + out_names
        if part_name is not None:
            self.in_names = self.in_names + [part_name]
        self.out_names = out_names
        self.out_avals = out_avals
        self.zero_outs = zero_outs

        def _body(*args):
            operands = list(args)
            if part_name is not None:
                operands.append(bass2jax.partition_id_tensor())
            outs = bass2jax._bass_exec_p.bind(
                *operands,
                out_avals=tuple(out_avals),
                in_names=tuple(self.in_names),
                out_names=tuple(out_names),
                lowering_input_output_aliases=(),
                sim_require_finite=True,
                sim_require_nnan=True,
                nc=nc,
            )
            return tuple(outs)

        devices = jax.devices()[:NCORES]
        self.mesh = Mesh(np.asarray(devices), ("core",))
        n_all = self.n_params + len(out_names)
        self.fn = jax.jit(
            shard_map(_body, mesh=self.mesh,
                      in_specs=(PartitionSpec("core"),) * n_all,
                      out_specs=(PartitionSpec("core"),) * len(out_names),
                      check_rep=False),
            keep_unused=True,
        )

    def prepare(self, in_maps):
        concat = [
            np.concatenate([np.asarray(m[nm]) for m in in_maps], axis=0)
            for nm in self.in_names[:self.n_params]
        ]
        concat += [
            np.zeros((NCORES * z.shape[0], *z.shape[1:]), z.dtype)
            for z in self.zero_outs
        ]
        return concat

    def run(self, args):
        outs = self.fn(*args)
        return [
            {nm: np.asarray(outs[i]).reshape(NCORES, *self.out_avals[i].shape)[c]
             for i, nm in enumerate(self.out_names)}
            for c in range(NCORES)
        ]


# ----------------------------------------------------------------------------
# Host-side sharding / index plumbing
# ----------------------------------------------------------------------------

def _round_up(v, m):
    return (v + m - 1) // m * m


def _pow2_round(v):
    p = 1
    while p < v:
        p *= 2
    return p


def prepare(x, group_indices, pool_cluster_fine, batch_cluster_coarse,
            W1, b1, W2, b2, w_out, b_out):
    """Compute capacities + per-core input maps. Returns (key, in_maps, meta)."""
    x = np.asarray(x)
    gidx = np.asarray(group_indices)
    pcf = np.asarray(pool_cluster_fine).astype(np.int64)
    bcc = np.asarray(batch_cluster_coarse).astype(np.int64)
    W1 = np.asarray(W1, dtype=np.float32)
    b1 = np.asarray(b1, dtype=np.float32)
    W2 = np.asarray(W2, dtype=np.float32)
    b2 = np.asarray(b2, dtype=np.float32)
    w_out = np.asarray(w_out, dtype=np.float32)
    b_out = np.asarray(b_out, dtype=np.float32)

    GPC = G_SEG // NCORES

    # node -> group (later groups win on duplicates, matching scatter order)
    gid = np.full(N, -1, np.int32)
    for g in range(NG):
        gid[gidx[g]] = g

    # graph/cluster/node boundaries (general sorted-segment support)
    fine_lo = np.searchsorted(bcc, np.arange(0, G_SEG, GPC))          # per core
    fine_hi = np.searchsorted(bcc, np.arange(GPC - 1, G_SEG, GPC), "right")
    node_lo = np.searchsorted(pcf, fine_lo)
    node_hi = np.searchsorted(pcf, fine_hi)

    # cluster boundaries for every fine cluster
    cl_lo = np.searchsorted(pcf, np.arange(F_SEG))
    cl_hi = np.searchsorted(pcf, np.arange(F_SEG), "right")
    cl_sz = cl_hi - cl_lo
    MCAP = _pow2_round(max(1, int(cl_sz.max())))

    # graph boundaries in cluster space, per core
    g_lo = np.searchsorted(bcc, np.arange(G_SEG))
    g_hi = np.searchsorted(bcc, np.arange(G_SEG), "right")
    g_sz = g_hi - g_lo
    CCAP = _round_up(max(1, int(g_sz.max())), P)

    # rows per (core, group)
    counts = np.zeros((NCORES, NG), np.int64)
    core_nodes = []
    for c in range(NCORES):
        nd = np.arange(node_lo[c], node_hi[c])
        core_nodes.append(nd)
        gs = gid[nd]
        for g in range(NG):
            counts[c, g] = int((gs == g).sum())
    GCAP = _round_up(max(1, int(counts.max())), CHUNK)
    RTOT = NG * GCAP
    assert 2 + RTOT < 32768, f"GCAP={GCAP} too large for int16 gather indices"
    SLOTS = GPC * CCAP
    NBLK = SLOTS // P

    # replicated weight prep (shared across cores)
    w1_h = np.ascontiguousarray(W1.transpose(1, 0, 2))                 # [128,4,1024]
    w2_h = np.ascontiguousarray(
        W2.reshape(NG, KEXP // P, P, H).transpose(2, 0, 1, 3))         # [128,4,8,512]
    b1_h = np.ascontiguousarray(
        b1.reshape(NG, KEXP // P, P).transpose(2, 0, 1).reshape(P, -1))  # [128,32]
    b2_h = np.ascontiguousarray(
        np.broadcast_to(b2[None, :, :], (P, NG, H)))                   # [128,4,512]
    wo_h = np.ascontiguousarray(
        w_out.reshape(H // P, P, C_CLS).transpose(1, 0, 2))            # [128,4,16]
    bo_h = np.ascontiguousarray(b_out.reshape(C_CLS, 1))               # [16,1]

    in_maps = []
    meta = []
    for c in range(NCORES):
        nd = core_nodes[c]
        gs = gid[nd]
        xt = np.zeros((P, RTOT), np.float32)
        rows = np.zeros(N, np.int32)     # node -> og row (0 = zero row)
        for g in range(NG):
            sel = nd[gs == g]
            cnt = len(sel)
            xt[:, g * GCAP:g * GCAP + cnt] = x[sel].T
            rows[sel] = 2 + g * GCAP + np.arange(cnt, dtype=np.int32)

        # member table: [SLOTS, MCAP] og-row indices
        member = np.ones((SLOTS, MCAP), np.int32)      # 1 = -inf row (pad member)
        clusters_c = np.arange(fine_lo[c], fine_hi[c])
        inv_cnt = np.zeros(GPC, np.float32)
        for gi in range(GPC):
            gg = c * GPC + gi
            n_cl = int(g_sz[gg])
            inv_cnt[gi] = 1.0 / max(n_cl, 1)
        for s, f in enumerate(clusters_c):
            gi = int(bcc[f]) - c * GPC
            j = f - g_lo[int(bcc[f])]
            slot = gi * CCAP + j
            sz = int(cl_sz[f])
            member[slot, :sz] = rows[pcf_nodes_start(cl_lo, f) +
                                     np.arange(sz)]
        # pad clusters: all-zero row (emb = 0 so norm sums are unaffected)
        used = np.zeros(SLOTS, bool)
        for f in clusters_c:
            gi = int(bcc[f]) - c * GPC
            slot = gi * CCAP + (f - g_lo[int(bcc[f])])
            used[slot] = True
        member[~used, :] = 0

        # wrap indices for dma_gather: per block t, seq i = m*128 + a
        gidx_w = np.zeros((P, NBLK * MCAP * P // 16), np.int16)
        for t in range(NBLK):
            mt = member[t * P:(t + 1) * P]               # [128, MCAP]
            seq = mt.T.reshape(-1)                        # i = m*128 + a
            w = seq.reshape(-1, 16).T.astype(np.int16)    # [16, MCAP*128/16]
            gidx_w[:, t * (MCAP * P // 16):(t + 1) * (MCAP * P // 16)] = \
                np.tile(w, (8, 1))

        in_maps.append({
            "xt": xt,
            "w1": w1_h, "w2": w2_h, "b1s": b1_h, "b2r": b2_h,
            "wout": wo_h, "bout": bo_h,
            "invc": np.broadcast_to(inv_cnt[None, :], (P, GPC)).copy(),
            "ident": np.eye(P, dtype=np.float32),
            "gidx": gidx_w,
        })
        meta.append({"clusters": clusters_c, "fine_lo": int(fine_lo[c]),
                     "g_lo": g_lo, "c": c})

    key = (GCAP, CCAP, MCAP)
    return key, in_maps, meta, (CCAP,)


def pcf_nodes_start(cl_lo, f):
    return int(cl_lo[f])


def get_runner(key):
    if key not in _PROGRAM_CACHE:
        nc = _build_program(*key)
        _PROGRAM_CACHE[key] = _Runner(nc)
    return _PROGRAM_CACHE[key]


def kernel(**inputs) -> np.ndarray:
    key, in_maps, meta, (CCAP,) = prepare(**inputs)
    runner = get_runner(key)
    args = runner.prepare(in_maps)
    results = runner.run(args)

    bcc = np.asarray(inputs["batch_cluster_coarse"]).astype(np.int64)
    GPC = G_SEG // NCORES
    g_lo = np.searchsorted(bcc, np.arange(G_SEG))
    out = np.zeros((F_SEG, C_CLS), np.float32)
    for c in range(NCORES):
        lo = results[c]["logt"]              # [16, SLOTS]
        for f in meta[c]["clusters"]:
            gi = int(bcc[f]) - c * GPC
            slot = gi * CCAP + (int(f) - int(g_lo[int(bcc[f])]))
            out[f] = lo[:, slot]
    return out
